# revision 1
# baseline (speedup 1.0000x reference)
"""Butterfly transform kernel for Trainium2 (8 NeuronCores, SPMD data parallel).

Math: reference applies 12 butterfly layers; every layer pairs the SAME
adjacent columns (2n, 2n+1) and multiplies each pair by a per-pair 2x2
matrix W[l, n].  The composition therefore collapses into a single per-pair
2x2 matrix  M[n] = W[0,n] @ W[1,n] @ ... @ W[11,n], so the device kernel is
ONE memory-bound pass over x instead of 12:

    y[:, 2n]   = x[:, 2n] * M[n,0,0] + x[:, 2n+1] * M[n,1,0]
    y[:, 2n+1] = x[:, 2n] * M[n,0,1] + x[:, 2n+1] * M[n,1,1]

Device layout: batch rows in SBUF partitions, features along free dim.
The folded weights are packed host-side into two full-width vectors
(A = diagonal terms, B = cross terms, interleaved per pair) so that
    y = x*A + pairswap(x*B),
three DVE tensor_tensor ops per [128, 4096] tile, with the pairswap
expressed purely as an access pattern (inner dim [2, step -1]).  The
weights are replicated across the 128 partitions once at startup by a
ones-matmul on the otherwise-idle PE.  Measured on trn2: ~104 us per pass
per core steady-state == the HBM roofline (32 MiB / ~358 GB/s shared
read+write, within ~10%); the DVE compute (~102 us) hides underneath.
"""

import sys
import numpy as np

if "/opt/trn_rl_repo" not in sys.path:
    sys.path.insert(0, "/opt/trn_rl_repo")

BATCH = 8192
SIZE = 4096
LOG_N = 12
HALF = SIZE // 2  # 2048
N_CORES = 8
ROWS_PER_CORE = BATCH // N_CORES  # 1024
P = 128  # SBUF partitions
N_TILES = ROWS_PER_CORE // P  # 8

_CACHE = {}


def _build_program(
    repeats: int = 1,
    mode: str = "swst",
    mul_v_dve_cols: int = SIZE,
    xio_bufs: int = 4,
    yio_bufs: int = 3,
):
    import concourse.bass as bass
    import concourse.bacc as bacc
    import concourse.mybir as mybir
    from concourse import tile
    from contextlib import ExitStack

    f32 = mybir.dt.float32
    nc = bacc.Bacc(
        None, num_swdge_queues=4 if mode in ("swst", "swstn", "copysw") else 1
    )

    x_in = nc.dram_tensor("x", [ROWS_PER_CORE, SIZE], f32, kind="ExternalInput")
    w_in = nc.dram_tensor("wf", [1, 2 * SIZE], f32, kind="ExternalInput")
    y_out = nc.dram_tensor("y", [ROWS_PER_CORE, SIZE], f32, kind="ExternalOutput")

    mult = mybir.AluOpType.mult
    add = mybir.AluOpType.add

    with tile.TileContext(nc) as tc, ExitStack() as ctx:
        const = ctx.enter_context(tc.tile_pool(name="const", bufs=1))
        xio = ctx.enter_context(tc.tile_pool(name="xio", bufs=xio_bufs))
        yio = ctx.enter_context(
            tc.tile_pool(name="yio", bufs=3 if mode == "full3s" else yio_bufs)
        )
        tmp = ctx.enter_context(tc.tile_pool(name="tmp", bufs=2))
        psum = ctx.enter_context(tc.tile_pool(name="psum", bufs=2, space="PSUM"))

        # Broadcast the 2 interleaved full-width weight vectors to all 128
        # partitions. One 32KB DMA brings the row into wb's partition 0; the
        # ones-matmul on the (idle) PE replicates it across partitions, and
        # the (idle) ACT engine copies PSUM->SBUF (overwriting partition 0
        # with the same values). Avoids re-reading 4MiB from HBM.
        ones = const.tile([1, P], f32)
        nc.vector.memset(ones[:], 1.0)
        wb = const.tile([P, 2 * SIZE], f32)
        wrow = wb[0:1, :]
        nc.sync.dma_start(wrow, w_in[:])
        for c in range(4):
            pt = psum.tile([P, HALF], f32, tag="wpsum")
            for j in range(HALF // 512):
                nc.tensor.matmul(
                    pt[:, j * 512 : (j + 1) * 512],
                    ones[:],
                    wrow[:, c * HALF + j * 512 : c * HALF + (j + 1) * 512],
                    start=True,
                    stop=True,
                )
            nc.scalar.copy(wb[:, c * HALF : (c + 1) * HALF], pt[:])
        a_full = wb[:, 0:SIZE]   # [A00 A11 A00 A11 ...] interleaved per pair
        b_full = wb[:, SIZE:]    # [A01 A10 A01 A10 ...] interleaved per pair

        for i in range(N_TILES * repeats):
            i = i % N_TILES
            rows = slice(i * P, (i + 1) * P)
            xt = xio.tile([P, SIZE], f32, tag="xt")
            if mode == "full3s":
                nc.sync.dma_start(xt[:, :HALF], x_in[rows, 0:HALF])
                nc.sync.dma_start(xt[:, HALF:], x_in[rows, HALF:])
            else:
                nc.sync.dma_start(xt[:], x_in[rows, :])

            yt = yio.tile([P, SIZE], f32, tag="yt")

            if mode == "copy":
                # DMA-roofline probe: no compute, store the loaded tile.
                nc.sync.dma_start(y_out[rows, :], xt[:])
                continue
            if mode == "copysw":
                # Roofline probe with stores on the SWDGE queues instead,
                # leaving all 8 HWDGE queues to the loads.
                nc.gpsimd.dma_start(y_out[rows, :], xt[:])
                continue
            if mode == "swst":
                nc.vector.tensor_tensor(yt[:], xt[:], a_full, mult)
                vts = tmp.tile([P, SIZE], f32, tag="vt")
                nc.vector.tensor_tensor(vts[:], xt[:], b_full, mult)
                y3 = yt[:].rearrange("p (n two) -> p n two", two=2)
                v3 = vts[:].rearrange("p (n two) -> p n two", two=2)[:, :, ::-1]
                nc.vector.tensor_tensor(y3, y3, v3, add)
                nc.gpsimd.dma_start(y_out[rows, :], yt[:])
                continue
            if mode == "swstn":
                # swst + vt-free (x*B computed in place into xt): the freed
                # 32KB/partition funds deeper load prefetch (xio_bufs=6).
                nc.vector.tensor_tensor(yt[:], xt[:], a_full, mult)
                nc.vector.tensor_tensor(xt[:], xt[:], b_full, mult)
                y3 = yt[:].rearrange("p (n two) -> p n two", two=2)
                v3 = xt[:].rearrange("p (n two) -> p n two", two=2)[:, :, ::-1]
                nc.vector.tensor_tensor(y3, y3, v3, add)
                nc.gpsimd.dma_start(y_out[rows, :], yt[:])
                continue

            # y[2n]   = x[2n]*M[n,0,0] + x[2n+1]*M[n,1,0]
            # y[2n+1] = x[2n]*M[n,0,1] + x[2n+1]*M[n,1,1]
            # u = x*A (diag terms), v = x*B (cross terms), y = u + pairswap(v)
            if mode == "full3n":
                # vt-free: reuse xt for x*B (xt is dead after the two muls),
                # freeing SBUF for deeper load prefetch.
                nc.vector.tensor_tensor(yt[:], xt[:], a_full, mult)
                nc.vector.tensor_tensor(xt[:], xt[:], b_full, mult)
                y3 = yt[:].rearrange("p (n two) -> p n two", two=2)
                v3 = xt[:].rearrange("p (n two) -> p n two", two=2)[:, :, ::-1]
                nc.vector.tensor_tensor(y3, y3, v3, add)
                nc.sync.dma_start(y_out[rows, :], yt[:])
                continue
            vt = tmp.tile([P, SIZE], f32, tag="vt")
            if mode in ("full3", "full3s"):
                nc.vector.tensor_tensor(yt[:], xt[:], a_full, mult)
                nc.vector.tensor_tensor(vt[:], xt[:], b_full, mult)
                y3 = yt[:].rearrange("p (n two) -> p n two", two=2)
                v3 = vt[:].rearrange("p (n two) -> p n two", two=2)[:, :, ::-1]
                nc.vector.tensor_tensor(y3, y3, v3, add)
            elif mode == "split":
                # Engine-balanced: DVE runs 2x-mode contiguous ops; GPSIMD
                # takes part of the v-mul; ACT does the pairswap copy.
                d = mul_v_dve_cols
                sw = tmp.tile([P, SIZE], f32, tag="sw")
                nc.vector.tensor_tensor(yt[:], xt[:], a_full, mult)
                if d > 0:
                    nc.vector.tensor_tensor(vt[:, :d], xt[:, :d], b_full[:, :d], mult)
                if d < SIZE:
                    nc.gpsimd.tensor_tensor(vt[:, d:], xt[:, d:], b_full[:, d:], mult)
                v3 = vt[:].rearrange("p (n two) -> p n two", two=2)[:, :, ::-1]
                s3 = sw[:].rearrange("p (n two) -> p n two", two=2)
                nc.scalar.copy(s3, v3)
                nc.vector.tensor_tensor(yt[:], yt[:], sw[:], add)
            else:
                raise ValueError(mode)

            if mode == "full3s":
                nc.sync.dma_start(y_out[rows, 0:HALF], yt[:, :HALF])
                nc.sync.dma_start(y_out[rows, HALF:], yt[:, HALF:])
            else:
                nc.sync.dma_start(y_out[rows, :], yt[:])

    nc.compile()
    return nc


def _get_nc(
    repeats: int = 1,
    mode: str = "swst",
    mul_v_dve_cols: int = SIZE,
    xio_bufs: int = 4,
    yio_bufs: int = 3,
):
    key = ("nc", repeats, mode, mul_v_dve_cols, xio_bufs, yio_bufs)
    if key not in _CACHE:
        _CACHE[key] = _build_program(repeats, mode, mul_v_dve_cols, xio_bufs, yio_bufs)
    return _CACHE[key]


def fold_weights(W: np.ndarray) -> np.ndarray:
    """Compose the 12 stacked per-pair 2x2 layers into one, in float64.

    Returns wf [1, 2*SIZE] float32: full-width A (diag: A00,A11 interleaved)
    followed by full-width B (cross: A01,A10 interleaved)."""
    Wd = W.astype(np.float64)  # [12, HALF, 2, 2]
    M = Wd[0]
    for l in range(1, Wd.shape[0]):
        M = np.einsum("nij,njk->nik", M, Wd[l])
    M = M.astype(np.float32)  # [HALF, 2, 2]
    a_full = np.stack([M[:, 0, 0], M[:, 1, 1]], axis=1).reshape(SIZE)
    b_full = np.stack([M[:, 0, 1], M[:, 1, 0]], axis=1).reshape(SIZE)
    wf = np.concatenate([a_full, b_full])
    return np.ascontiguousarray(wf.reshape(1, 2 * SIZE))


def _run(x: np.ndarray, W: np.ndarray, **run_kwargs):
    """Shard, run on the 8 cores, gather. Returns (output, BassKernelResults)."""
    from concourse.bass_utils import run_bass_kernel_spmd

    assert x.shape == (BATCH, SIZE) and W.shape == (LOG_N, HALF, 2, 2)
    x = np.ascontiguousarray(x, dtype=np.float32)
    wf = fold_weights(np.asarray(W))

    nc = _get_nc()
    in_maps = [
        {"x": x[c * ROWS_PER_CORE : (c + 1) * ROWS_PER_CORE], "wf": wf}
        for c in range(N_CORES)
    ]
    res = run_bass_kernel_spmd(nc, in_maps, core_ids=list(range(N_CORES)), **run_kwargs)
    out = np.concatenate([res.results[c]["y"] for c in range(N_CORES)], axis=0)
    return out, res


def kernel(x: np.ndarray, W: np.ndarray) -> np.ndarray:
    return _run(x, W)[0]

